# revision 25
# baseline (speedup 1.0000x reference)
"""Trainium2 Bass kernel for CapsNet dynamic routing (nn_Capsule_13692355740297).

Math (per batch element):
    u_hat[i, (n,d)] = u[i, :] @ W[:, (n,d)]            # never materialized
    iter1: c uniform 1/10  -> s1 = 0.1 * (sum_i u_i)^T W
    iter k: b[i, n] = v_n . u_i   with v_n = W_n o_n   # contract Din on PE
            c = softmax_n(b)                           # free-dim softmax, [i,n] layout
            R[n, :] = sum_i c[i, n] u_i                # contract i on PE
            s[n, :] = R[n, :] @ W_n                    # small fixup matmuls
            o = squash(s)
Sharding: data-parallel over batch, 8 batch elements per core, no collectives.

v4: batches run in 4 independent groups of 2 whose routing overlaps the HBM
load of later batches.  The Tile scheduler's cost model does not model HBM
contention (it thinks the u loads land in a few us), so the static per-engine
order it produces is essentially program order; stages are therefore emitted
in an order hand-matched to the TRUE load timeline, with next-batch
transposes filling the current group's softmax/squash stalls.  squash's sqrt
runs on the vector engine (rsqrt bit-hack + Newton) so the ACT engine only
ever needs the exp table set (act-table reloads cost ~2.7us each).
"""

import numpy as np

B, I_FULL, DIN = 64, 4096, 128
NCAP, DCAP = 10, 16
KND = NCAP * DCAP  # 160
NCORES = 8
BC = B // NCORES  # 8 batch elements per core
NT_FULL = I_FULL // 128  # 32 i-tiles per batch
EPS = 1e-7
GB = 2  # batches per group
GROUPS = [list(range(g, g + GB)) for g in range(0, BC, GB)]


def build_nc(bc=BC, nt=NT_FULL):
    import concourse.bacc as bacc
    import concourse.mybir as mybir
    from concourse.tile import TileContext

    fp32 = mybir.dt.float32
    bf16 = mybir.dt.bfloat16
    AX = mybir.AxisListType
    ALU = mybir.AluOpType
    ACTF = mybir.ActivationFunctionType

    il = nt * 128  # I per batch

    nc = bacc.Bacc(trn_type="TRN2")
    u_h = nc.dram_tensor("u", [bc, il, DIN], fp32, kind="ExternalInput")
    w_h = nc.dram_tensor("w", [DIN, KND], fp32, kind="ExternalInput")
    ident_h = nc.dram_tensor("ident", [128, 128], fp32, kind="ExternalInput")
    identb_h = nc.dram_tensor("identb", [128, 128], bf16, kind="ExternalInput")
    wt_hi_h = nc.dram_tensor("wt_hi", [128, DIN], bf16, kind="ExternalInput")
    wt_lo_h = nc.dram_tensor("wt_lo", [32, DIN], bf16, kind="ExternalInput")
    m_hi_h = nc.dram_tensor("m_hi", [128, NCAP], bf16, kind="ExternalInput")
    m_lo_h = nc.dram_tensor("m_lo", [32, NCAP], bf16, kind="ExternalInput")
    esel_h = nc.dram_tensor("esel", [128, GB * GB], fp32, kind="ExternalInput")
    out_h = nc.dram_tensor("out", [bc, KND], fp32, kind="ExternalOutput")

    with TileContext(nc) as tc:
        with (
            tc.tile_pool(name="big", bufs=1) as big,
            tc.tile_pool(name="sb2", bufs=3) as sb2,
            tc.tile_pool(name="psT", bufs=2, space="PSUM") as psT,
            tc.tile_pool(name="psB", bufs=2, space="PSUM") as psB,
            tc.tile_pool(name="psR", bufs=2, space="PSUM") as psR,
            tc.tile_pool(name="psS", bufs=2, space="PSUM") as psS,
        ):
            # ---------- persistent SBUF ----------
            U_b = [big.tile([128, il], bf16, name=f"U_sb{b}") for b in range(bc)]
            UT_b = [big.tile([128, il], bf16, name=f"UT_sb{b}") for b in range(bc)]
            W_sb = big.tile([128, KND], fp32, name="W_sb")
            ident = big.tile([128, 128], fp32, name="ident_sb")
            identb = big.tile([128, 128], bf16, name="identb_sb")
            wt_hi = big.tile([128, DIN], bf16, name="wt_hi_sb")
            wt_lo = big.tile([32, DIN], bf16, name="wt_lo_sb")
            m_hi = big.tile([128, NCAP], bf16, name="m_hi_sb")
            m_lo = big.tile([32, NCAP], bf16, name="m_lo_sb")
            esel = big.tile([128, GB * GB], fp32, name="esel_sb")
            r0 = big.tile([128, bc], fp32, name="r0_sb")

            Uv = [
                U_b[b][:, :].rearrange("p (m d) -> p m d", m=nt, d=128)
                for b in range(bc)
            ]
            Wv = W_sb[:, :].rearrange("p (n d) -> p n d", n=NCAP)

            # ---------- u loads first: 16KB contiguous per partition ----------
            hh = nt // 2
            for b in range(bc):
                uin = u_h.ap()[b].rearrange("(p m) d -> p m d", m=nt)
                nc.gpsimd.dma_start(out=Uv[b][:, :hh], in_=uin[:, :hh])
                nc.gpsimd.dma_start(out=Uv[b][:, hh:], in_=uin[:, hh:])

            # ---------- consts on the HWDGE path (parallel with u DGE) ----------
            nc.sync.dma_start(out=W_sb[:, :], in_=w_h.ap())
            nc.sync.dma_start(out=ident[:, :], in_=ident_h.ap())
            nc.sync.dma_start(out=identb[:, :], in_=identb_h.ap())
            nc.sync.dma_start(out=wt_hi[:, :], in_=wt_hi_h.ap())
            nc.sync.dma_start(out=wt_lo[:, :], in_=wt_lo_h.ap())
            nc.sync.dma_start(out=m_hi[:, :], in_=m_hi_h.ap())
            nc.sync.dma_start(out=m_lo[:, :], in_=m_lo_h.ap())
            nc.sync.dma_start(out=esel[:, :], in_=esel_h.ap())

            # ---------- per-batch: UT via PE matmul-transposes, r0 via copy-accumulators ----------
            # regular matmul (identity moving) instead of transpose-mode:
            # the stationary U_j load then qualifies for Fast Weight Load
            # (128 bf16 cols), about 2x cheaper on the PE weight path
            TG = 4
            ng = nt // TG

            def phase1(b):
                racc = sb2.tile([128, ng], fp32, name=f"racc{b}", tag="racc")
                for g in range(ng):
                    tp = psT.tile([128, TG * 128], fp32, name="tp", tag="tp")
                    for jj in range(TG):
                        nc.tensor.matmul(
                            tp[:, 128 * jj : 128 * (jj + 1)],
                            Uv[b][:, TG * g + jj],
                            identb[:, :],
                        )
                    dst = UT_b[b][:, TG * 128 * g : TG * 128 * (g + 1)]
                    if g % 2 == 0:
                        nc.scalar.activation(
                            dst, tp[:, :], ACTF.Copy, accum_out=racc[:, g : g + 1]
                        )
                    else:
                        nc.vector.tensor_scalar(
                            out=dst, in0=tp[:, :],
                            scalar1=0.0, scalar2=0.0,
                            op0=ALU.add, op1=ALU.add,
                            accum_out=racc[:, g : g + 1],
                        )
                nc.vector.reduce_sum(
                    out=r0[:, b : b + 1], in_=racc[:, :], axis=AX.X, op=ALU.add
                )

            def squash(s_g, it, gi):
                """s_g: [GB, KND] sbuf tile -> o_g [GB, KND].

                sqrt via rsqrt bit-hack + one Newton step on DVE; square on
                ACT (Square is in the exp table set, so no table reload).
                """
                i32 = mybir.dt.int32
                sq = sb2.tile([GB, KND], fp32, name=f"sq{gi}_{it}", tag="sq")
                q = sb2.tile([GB, NCAP], fp32, name=f"q{gi}_{it}", tag="q")
                h = sb2.tile([GB, NCAP], fp32, name=f"h{gi}_{it}", tag="h")
                y0 = sb2.tile([GB, NCAP], fp32, name=f"y0{gi}_{it}", tag="y0")
                y1 = sb2.tile([GB, NCAP], fp32, name=f"y1{gi}_{it}", tag="y1")
                a = sb2.tile([GB, NCAP], fp32, name=f"a{gi}_{it}", tag="aa")
                c = sb2.tile([GB, NCAP], fp32, name=f"c{gi}_{it}", tag="cc2")
                rt = sb2.tile([GB, NCAP], fp32, name=f"rt{gi}_{it}", tag="rt")
                den = sb2.tile([GB, NCAP], fp32, name=f"den{gi}_{it}", tag="den")
                rden = sb2.tile([GB, NCAP], fp32, name=f"rden{gi}_{it}", tag="rden")
                coef = sb2.tile([GB, NCAP], fp32, name=f"coef{gi}_{it}", tag="coef")
                o_g = sb2.tile([GB, KND], fp32, name=f"o{gi}_{it}", tag="og")
                nc.scalar.square(out=sq[:, :], in_=s_g[:, :])
                nc.vector.reduce_sum(
                    out=q[:, :],
                    in_=sq[:, :].rearrange("b (n d) -> b n d", n=NCAP),
                    axis=AX.X, op=ALU.add,
                )
                nc.vector.tensor_scalar(
                    out=h[:, :].bitcast(i32), in0=q[:, :].bitcast(i32),
                    scalar1=1, scalar2=None, op0=ALU.arith_shift_right,
                )
                nc.vector.tensor_scalar(
                    out=y0[:, :].bitcast(i32), in0=h[:, :].bitcast(i32),
                    scalar1=-1, scalar2=0x5F3759DF, op0=ALU.mult, op1=ALU.add,
                )
                nc.vector.tensor_tensor(
                    out=a[:, :], in0=y0[:, :], in1=y0[:, :], op=ALU.mult
                )
                nc.vector.tensor_tensor(
                    out=a[:, :], in0=a[:, :], in1=q[:, :], op=ALU.mult
                )
                nc.vector.tensor_scalar(
                    out=c[:, :], in0=a[:, :],
                    scalar1=-0.5, scalar2=1.5, op0=ALU.mult, op1=ALU.add,
                )
                nc.vector.tensor_tensor(
                    out=y1[:, :], in0=y0[:, :], in1=c[:, :], op=ALU.mult
                )
                nc.vector.tensor_tensor(
                    out=rt[:, :], in0=q[:, :], in1=y1[:, :], op=ALU.mult
                )
                nc.vector.tensor_scalar_add(den[:, :], q[:, :], 1.0)
                nc.vector.reciprocal(out=rden[:, :], in_=den[:, :])
                nc.vector.tensor_tensor(
                    out=coef[:, :], in0=rt[:, :], in1=rden[:, :], op=ALU.mult
                )
                nc.vector.tensor_tensor(
                    out=o_g[:, :].rearrange("b (n d) -> b n d", n=NCAP),
                    in0=s_g[:, :].rearrange("b (n d) -> b n d", n=NCAP),
                    in1=coef[:, :].unsqueeze(2).broadcast_to([GB, NCAP, DCAP]),
                    op=ALU.mult,
                )
                return o_g

            def make_V(o_g, it, gi):
                oth_p = psT.tile([128, GB], fp32, name=f"oth{gi}_{it}", tag="tp")
                otl_p = psT.tile([32, GB], fp32, name=f"otl{gi}_{it}", tag="tp")
                nc.tensor.transpose(oth_p[:, :], o_g[:, 0:128], ident[:GB, :GB])
                nc.tensor.transpose(otl_p[:, :], o_g[:, 128:KND], ident[:GB, :GB])
                # oeh/oel read the transposed o straight from PSUM (DVE can),
                # skipping two ACT copies on the critical chain
                oeh = sb2.tile([128, GB * NCAP], bf16, name=f"oeh{gi}_{it}", tag="oeh")
                oel = sb2.tile([32, GB * NCAP], bf16, name=f"oel{gi}_{it}", tag="oel")
                nc.vector.tensor_tensor(
                    out=oeh[:, :].rearrange("p (b n) -> p b n", b=GB),
                    in0=oth_p[:, :].unsqueeze(2).broadcast_to([128, GB, NCAP]),
                    in1=m_hi[:, :].unsqueeze(1).broadcast_to([128, GB, NCAP]),
                    op=ALU.mult,
                )
                nc.vector.tensor_tensor(
                    out=oel[:, :].rearrange("p (b n) -> p b n", b=GB),
                    in0=otl_p[:, :].unsqueeze(2).broadcast_to([32, GB, NCAP]),
                    in1=m_lo[:, :].unsqueeze(1).broadcast_to([32, GB, NCAP]),
                    op=ALU.mult,
                )
                vp = psT.tile([128, GB * NCAP], fp32, name=f"vp{gi}_{it}", tag="tp")
                nc.tensor.matmul(vp[:, :], wt_hi[:, :], oeh[:, :], start=True, stop=False)
                nc.tensor.matmul(vp[:, :], wt_lo[:, :], oel[:, :], start=False, stop=True)
                V = sb2.tile([128, GB * NCAP], bf16, name=f"V{gi}_{it}", tag="V")
                nc.vector.tensor_copy(out=V[:, :], in_=vp[:, :])
                return V

            # ---------- staged group pipeline, hand-interleaved emission ----------
            st = {gi: {} for gi in range(len(GROUPS))}

            def chainA(gi):
                # iter 1: s = 0.1 * r0_g^T W, squash, V for iter 2
                g0 = GROUPS[gi][0]
                s1p = psS.tile([GB, KND], fp32, name=f"s1p{gi}", tag="sacc")
                nc.tensor.matmul(s1p[:, :], r0[:, g0 : g0 + GB], W_sb[:, :])
                s_g = sb2.tile([GB, KND], fp32, name=f"s1_{gi}", tag="sg")
                nc.vector.tensor_scalar_mul(s_g[:, :], s1p[:, :], 0.1)
                o_g = squash(s_g, 1, gi)
                st[gi]["V"] = make_V(o_g, 2, gi)

            def bp_sfmx(gi, it):
                batches = GROUPS[gi]
                g0 = batches[0]
                V = st[gi]["V"]
                btps = {}
                for b in batches:
                    btp = psB.tile([128, nt * NCAP], fp32, name=f"btp{gi}_{it}_{b}", tag="btp")
                    bl = b - g0
                    for j in range(nt):
                        nc.tensor.matmul(
                            btp[:, NCAP * j : NCAP * (j + 1)],
                            UT_b[b][:, 128 * j : 128 * (j + 1)],
                            V[:, NCAP * bl : NCAP * (bl + 1)],
                        )
                    btps[b] = btp
                ccs = {}
                for b in batches:
                    eb = sb2.tile([128, nt * NCAP], fp32, name=f"eb{it}_{b}", tag="eb")
                    nc.scalar.activation(eb[:, :], btps[b][:, :], ACTF.Exp)
                    ebv = eb[:, :].rearrange("p (j n) -> p j n", j=nt)
                    # Z and cc on GpSimd: it is idle once the load DGE work is
                    # done, while DVE is the post-load bottleneck engine
                    Z = sb2.tile([128, nt], fp32, name=f"Z{it}_{b}", tag="Z")
                    nc.vector.reduce_sum(out=Z[:, :], in_=ebv, axis=AX.X, op=ALU.add)
                    rZ = sb2.tile([128, nt], fp32, name=f"rZ{it}_{b}", tag="rZ")
                    nc.vector.reciprocal(out=rZ[:, :], in_=Z[:, :])
                    cc = sb2.tile([128, nt * NCAP], bf16, name=f"cc{it}_{b}", tag="cc")
                    nc.gpsimd.tensor_tensor(
                        out=cc[:, :].rearrange("p (j n) -> p j n", j=nt),
                        in0=ebv,
                        in1=rZ[:, :].unsqueeze(2).broadcast_to([128, nt, NCAP]),
                        op=ALU.mult,
                    )
                    ccs[b] = cc
                st[gi]["ccs"] = ccs

            def Rpass(gi, it):
                batches = GROUPS[gi]
                g0 = batches[0]
                ccs = st[gi]["ccs"]
                sp = psS.tile([GB, KND], fp32, name=f"sp{gi}_{it}", tag="sacc")
                for b in batches:
                    bl = b - g0
                    cc = ccs[b]
                    Rp = psR.tile([128, NCAP], fp32, name=f"Rp{it}_{b}", tag="Rp")
                    for j in range(nt):
                        nc.tensor.matmul(
                            Rp[:, :],
                            Uv[b][:, j],
                            cc[:, NCAP * j : NCAP * (j + 1)],
                            start=(j == 0),
                            stop=(j == nt - 1),
                        )
                    prod = sb2.tile([128, KND], fp32, name=f"prod{it}_{b}", tag="prod")
                    nc.vector.tensor_tensor(
                        out=prod[:, :].rearrange("p (n d) -> p n d", n=NCAP),
                        in0=Rp[:, :].unsqueeze(2).broadcast_to([128, NCAP, DCAP]),
                        in1=Wv,
                        op=ALU.mult,
                    )
                    nc.tensor.matmul(
                        sp[:, :],
                        esel[:, GB * bl : GB * (bl + 1)],
                        prod[:, :],
                        start=(bl == 0),
                        stop=(bl == GB - 1),
                    )
                s_g = sb2.tile([GB, KND], fp32, name=f"s{gi}_{it}", tag="sg")
                nc.vector.tensor_copy(out=s_g[:, :], in_=sp[:, :])
                st[gi]["s"] = s_g

            def chainB(gi, it):
                o_g = squash(st[gi]["s"], it, gi)
                st[gi]["V"] = make_V(o_g, it + 1, gi)

            def chainC(gi):
                g0 = GROUPS[gi][0]
                o_g = squash(st[gi]["s"], 3, gi)
                nc.sync.dma_start(out=out_h.ap()[g0 : g0 + GB, :], in_=o_g[:, :])

            P, A, CC = phase1, chainA, chainC
            CB = lambda g: chainB(g, 2)
            B2 = lambda g: bp_sfmx(g, 2)
            B3 = lambda g: bp_sfmx(g, 3)
            R2 = lambda g: Rpass(g, 2)
            R3 = lambda g: Rpass(g, 3)
            emit = [
                (P, 0), (P, 1),
                (A, 0),
                (B2, 0), (P, 2), (R2, 0), (P, 3), (CB, 0),
                (B3, 0), (A, 1), (R3, 0), (CC, 0),
                (B2, 1), (P, 4), (R2, 1), (P, 5), (CB, 1),
                (B3, 1), (A, 2), (R3, 1), (CC, 1),
                (B2, 2), (P, 6), (R2, 2), (P, 7), (CB, 2), (A, 3),
                (B3, 2), (B2, 3), (R3, 2), (R2, 3), (CC, 2),
                (CB, 3), (B3, 3), (R3, 3), (CC, 3),
            ]
            for fn, arg in emit:
                fn(arg)

    nc.compile()
    return nc


def make_const_inputs():
    import ml_dtypes

    ident = np.eye(128, dtype=np.float32)
    mask = np.zeros((KND, NCAP), dtype=np.float32)
    for k in range(KND):
        mask[k, k // DCAP] = 1.0
    esel = np.zeros((128, GB * GB), dtype=np.float32)
    for b in range(GB):
        esel[:, b * GB + b] = 1.0
    return {
        "ident": ident,
        "identb": ident.astype(ml_dtypes.bfloat16),
        "m_hi": mask[:128].astype(ml_dtypes.bfloat16),
        "m_lo": mask[128:].astype(ml_dtypes.bfloat16),
        "esel": esel,
    }


def make_w_inputs(W):
    import ml_dtypes

    W = np.asarray(W, dtype=np.float32)
    WT = W.T.copy()  # [160, 128]
    return {
        "w": W,
        "wt_hi": WT[:128].astype(ml_dtypes.bfloat16),
        "wt_lo": WT[128:].astype(ml_dtypes.bfloat16),
    }


_CACHE = {}


def kernel(u_vecs, W):
    from concourse import bass_utils

    u_vecs = np.asarray(u_vecs, dtype=np.float32)
    W = np.asarray(W, dtype=np.float32)
    if "nc" not in _CACHE:
        _CACHE["nc"] = build_nc()
    nc = _CACHE["nc"]

    consts = make_const_inputs()
    wis = make_w_inputs(W)
    in_maps = []
    for c in range(NCORES):
        m = {"u": np.ascontiguousarray(u_vecs[c * BC : (c + 1) * BC])}
        m.update(consts)
        m.update(wis)
        in_maps.append(m)

    res = bass_utils.run_bass_kernel_spmd(nc, in_maps, core_ids=list(range(NCORES)))
    outs = [r["out"] for r in res.results]
    return np.concatenate(outs, axis=0).reshape(B, NCAP, DCAP).astype(np.float32)


# revision 27
# speedup vs baseline: 1.0359x; 1.0359x over previous
"""Trainium2 Bass kernel for CapsNet dynamic routing (nn_Capsule_13692355740297).

Math (per batch element):
    u_hat[i, (n,d)] = u[i, :] @ W[:, (n,d)]            # never materialized
    iter1: c uniform 1/10  -> s1 = 0.1 * (sum_i u_i)^T W
    iter k: b[i, n] = v_n . u_i   with v_n = W_n o_n   # contract Din on PE
            c = softmax_n(b)                           # free-dim softmax, [i,n] layout
            R[n, :] = sum_i c[i, n] u_i                # contract i on PE
            s[n, :] = R[n, :] @ W_n                    # small fixup matmuls
            o = squash(s)
Sharding: data-parallel over batch, 8 batch elements per core, no collectives.

v4: batches run in 4 independent groups of 2 whose routing overlaps the HBM
load of later batches.  The Tile scheduler's cost model does not model HBM
contention (it thinks the u loads land in a few us), so the static per-engine
order it produces is essentially program order; stages are therefore emitted
in an order hand-matched to the TRUE load timeline, with next-batch
transposes filling the current group's softmax/squash stalls.  squash's sqrt
runs on the vector engine (rsqrt bit-hack + Newton) so the ACT engine only
ever needs the exp table set (act-table reloads cost ~2.7us each).
"""

import numpy as np

B, I_FULL, DIN = 64, 4096, 128
NCAP, DCAP = 10, 16
KND = NCAP * DCAP  # 160
NCORES = 8
BC = B // NCORES  # 8 batch elements per core
NT_FULL = I_FULL // 128  # 32 i-tiles per batch
EPS = 1e-7
GB = 2  # batches per group
GROUPS = [list(range(g, g + GB)) for g in range(0, BC, GB)]


def build_nc(bc=BC, nt=NT_FULL):
    import concourse.bacc as bacc
    import concourse.mybir as mybir
    from concourse.tile import TileContext

    fp32 = mybir.dt.float32
    bf16 = mybir.dt.bfloat16
    AX = mybir.AxisListType
    ALU = mybir.AluOpType
    ACTF = mybir.ActivationFunctionType

    il = nt * 128  # I per batch

    nc = bacc.Bacc(trn_type="TRN2")
    u_h = nc.dram_tensor("u", [bc, il, DIN], fp32, kind="ExternalInput")
    w_h = nc.dram_tensor("w", [DIN, KND], fp32, kind="ExternalInput")
    ident_h = nc.dram_tensor("ident", [128, 128], fp32, kind="ExternalInput")
    identb_h = nc.dram_tensor("identb", [128, 128], bf16, kind="ExternalInput")
    wt_hi_h = nc.dram_tensor("wt_hi", [128, DIN], bf16, kind="ExternalInput")
    wt_lo_h = nc.dram_tensor("wt_lo", [32, DIN], bf16, kind="ExternalInput")
    m_hi_h = nc.dram_tensor("m_hi", [128, NCAP], bf16, kind="ExternalInput")
    m_lo_h = nc.dram_tensor("m_lo", [32, NCAP], bf16, kind="ExternalInput")
    esel_h = nc.dram_tensor("esel", [128, GB * GB], bf16, kind="ExternalInput")
    out_h = nc.dram_tensor("out", [bc, KND], fp32, kind="ExternalOutput")

    with TileContext(nc) as tc:
        with (
            tc.tile_pool(name="big", bufs=1) as big,
            tc.tile_pool(name="sb2", bufs=3) as sb2,
            tc.tile_pool(name="psT", bufs=2, space="PSUM") as psT,
            tc.tile_pool(name="psB", bufs=2, space="PSUM") as psB,
            tc.tile_pool(name="psR", bufs=2, space="PSUM") as psR,
            tc.tile_pool(name="psS", bufs=2, space="PSUM") as psS,
        ):
            # ---------- persistent SBUF ----------
            U_b = [big.tile([128, il], bf16, name=f"U_sb{b}") for b in range(bc)]
            UT_b = [big.tile([128, il], bf16, name=f"UT_sb{b}") for b in range(bc)]
            W_sb = big.tile([128, KND], fp32, name="W_sb")
            ident = big.tile([128, 128], fp32, name="ident_sb")
            identb = big.tile([128, 128], bf16, name="identb_sb")
            wt_hi = big.tile([128, DIN], bf16, name="wt_hi_sb")
            wt_lo = big.tile([32, DIN], bf16, name="wt_lo_sb")
            m_hi = big.tile([128, NCAP], bf16, name="m_hi_sb")
            m_lo = big.tile([32, NCAP], bf16, name="m_lo_sb")
            esel = big.tile([128, GB * GB], bf16, name="esel_sb")
            r0 = big.tile([128, bc], fp32, name="r0_sb")

            Uv = [
                U_b[b][:, :].rearrange("p (m d) -> p m d", m=nt, d=128)
                for b in range(bc)
            ]
            Wv = W_sb[:, :].rearrange("p (n d) -> p n d", n=NCAP)

            # ---------- u loads first: 16KB contiguous per partition ----------
            hh = nt // 2
            for b in range(bc):
                uin = u_h.ap()[b].rearrange("(p m) d -> p m d", m=nt)
                nc.gpsimd.dma_start(out=Uv[b][:, :hh], in_=uin[:, :hh])
                nc.gpsimd.dma_start(out=Uv[b][:, hh:], in_=uin[:, hh:])

            # ---------- consts on the HWDGE path (parallel with u DGE) ----------
            nc.sync.dma_start(out=W_sb[:, :], in_=w_h.ap())
            nc.sync.dma_start(out=ident[:, :], in_=ident_h.ap())
            nc.sync.dma_start(out=identb[:, :], in_=identb_h.ap())
            nc.sync.dma_start(out=wt_hi[:, :], in_=wt_hi_h.ap())
            nc.sync.dma_start(out=wt_lo[:, :], in_=wt_lo_h.ap())
            nc.sync.dma_start(out=m_hi[:, :], in_=m_hi_h.ap())
            nc.sync.dma_start(out=m_lo[:, :], in_=m_lo_h.ap())
            nc.sync.dma_start(out=esel[:, :], in_=esel_h.ap())

            # ---------- per-batch: UT via PE matmul-transposes, r0 via copy-accumulators ----------
            # regular matmul (identity moving) instead of transpose-mode:
            # the stationary U_j load then qualifies for Fast Weight Load
            # (128 bf16 cols), about 2x cheaper on the PE weight path
            TG = 4
            ng = nt // TG

            def phase1(b):
                racc = sb2.tile([128, ng], fp32, name=f"racc{b}", tag="racc")
                for g in range(ng):
                    tp = psT.tile([128, TG * 128], fp32, name="tp", tag="tp")
                    for jj in range(TG):
                        nc.tensor.matmul(
                            tp[:, 128 * jj : 128 * (jj + 1)],
                            Uv[b][:, TG * g + jj],
                            identb[:, :],
                        )
                    dst = UT_b[b][:, TG * 128 * g : TG * 128 * (g + 1)]
                    if g % 2 == 0:
                        nc.scalar.activation(
                            dst, tp[:, :], ACTF.Copy, accum_out=racc[:, g : g + 1]
                        )
                    else:
                        nc.vector.tensor_scalar(
                            out=dst, in0=tp[:, :],
                            scalar1=0.0, scalar2=0.0,
                            op0=ALU.add, op1=ALU.add,
                            accum_out=racc[:, g : g + 1],
                        )
                nc.vector.reduce_sum(
                    out=r0[:, b : b + 1], in_=racc[:, :], axis=AX.X, op=ALU.add
                )

            def squash(s_g, it, gi):
                """s_g: [GB, KND] sbuf tile -> o_g [GB, KND].

                sqrt via rsqrt bit-hack + one Newton step on DVE; square on
                ACT (Square is in the exp table set, so no table reload).
                """
                i32 = mybir.dt.int32
                sq = sb2.tile([GB, KND], fp32, name=f"sq{gi}_{it}", tag="sq")
                q = sb2.tile([GB, NCAP], fp32, name=f"q{gi}_{it}", tag="q")
                h = sb2.tile([GB, NCAP], fp32, name=f"h{gi}_{it}", tag="h")
                y0 = sb2.tile([GB, NCAP], fp32, name=f"y0{gi}_{it}", tag="y0")
                y1 = sb2.tile([GB, NCAP], fp32, name=f"y1{gi}_{it}", tag="y1")
                a = sb2.tile([GB, NCAP], fp32, name=f"a{gi}_{it}", tag="aa")
                c = sb2.tile([GB, NCAP], fp32, name=f"c{gi}_{it}", tag="cc2")
                rt = sb2.tile([GB, NCAP], fp32, name=f"rt{gi}_{it}", tag="rt")
                den = sb2.tile([GB, NCAP], fp32, name=f"den{gi}_{it}", tag="den")
                rden = sb2.tile([GB, NCAP], fp32, name=f"rden{gi}_{it}", tag="rden")
                coef = sb2.tile([GB, NCAP], fp32, name=f"coef{gi}_{it}", tag="coef")
                o_g = sb2.tile([GB, KND], fp32, name=f"o{gi}_{it}", tag="og")
                nc.scalar.square(out=sq[:, :], in_=s_g[:, :])
                nc.vector.reduce_sum(
                    out=q[:, :],
                    in_=sq[:, :].rearrange("b (n d) -> b n d", n=NCAP),
                    axis=AX.X, op=ALU.add,
                )
                nc.vector.tensor_scalar(
                    out=h[:, :].bitcast(i32), in0=q[:, :].bitcast(i32),
                    scalar1=1, scalar2=None, op0=ALU.arith_shift_right,
                )
                nc.vector.tensor_scalar(
                    out=y0[:, :].bitcast(i32), in0=h[:, :].bitcast(i32),
                    scalar1=-1, scalar2=0x5F3759DF, op0=ALU.mult, op1=ALU.add,
                )
                nc.vector.tensor_tensor(
                    out=a[:, :], in0=y0[:, :], in1=y0[:, :], op=ALU.mult
                )
                nc.vector.tensor_tensor(
                    out=a[:, :], in0=a[:, :], in1=q[:, :], op=ALU.mult
                )
                nc.vector.tensor_scalar(
                    out=c[:, :], in0=a[:, :],
                    scalar1=-0.5, scalar2=1.5, op0=ALU.mult, op1=ALU.add,
                )
                nc.vector.tensor_tensor(
                    out=y1[:, :], in0=y0[:, :], in1=c[:, :], op=ALU.mult
                )
                nc.vector.tensor_tensor(
                    out=rt[:, :], in0=q[:, :], in1=y1[:, :], op=ALU.mult
                )
                nc.vector.tensor_scalar_add(den[:, :], q[:, :], 1.0)
                nc.vector.reciprocal(out=rden[:, :], in_=den[:, :])
                nc.vector.tensor_tensor(
                    out=coef[:, :], in0=rt[:, :], in1=rden[:, :], op=ALU.mult
                )
                nc.vector.tensor_tensor(
                    out=o_g[:, :].rearrange("b (n d) -> b n d", n=NCAP),
                    in0=s_g[:, :].rearrange("b (n d) -> b n d", n=NCAP),
                    in1=coef[:, :].unsqueeze(2).broadcast_to([GB, NCAP, DCAP]),
                    op=ALU.mult,
                )
                return o_g

            def make_V(o_g, it, gi):
                oth_p = psT.tile([128, GB], fp32, name=f"oth{gi}_{it}", tag="tp")
                otl_p = psT.tile([32, GB], fp32, name=f"otl{gi}_{it}", tag="tp")
                nc.tensor.transpose(oth_p[:, :], o_g[:, 0:128], ident[:GB, :GB])
                nc.tensor.transpose(otl_p[:, :], o_g[:, 128:KND], ident[:GB, :GB])
                # oeh/oel read the transposed o straight from PSUM (DVE can),
                # skipping two ACT copies on the critical chain
                oeh = sb2.tile([128, GB * NCAP], bf16, name=f"oeh{gi}_{it}", tag="oeh")
                oel = sb2.tile([32, GB * NCAP], bf16, name=f"oel{gi}_{it}", tag="oel")
                nc.vector.tensor_tensor(
                    out=oeh[:, :].rearrange("p (b n) -> p b n", b=GB),
                    in0=oth_p[:, :].unsqueeze(2).broadcast_to([128, GB, NCAP]),
                    in1=m_hi[:, :].unsqueeze(1).broadcast_to([128, GB, NCAP]),
                    op=ALU.mult,
                )
                nc.vector.tensor_tensor(
                    out=oel[:, :].rearrange("p (b n) -> p b n", b=GB),
                    in0=otl_p[:, :].unsqueeze(2).broadcast_to([32, GB, NCAP]),
                    in1=m_lo[:, :].unsqueeze(1).broadcast_to([32, GB, NCAP]),
                    op=ALU.mult,
                )
                vp = psT.tile([128, GB * NCAP], fp32, name=f"vp{gi}_{it}", tag="tp")
                nc.tensor.matmul(vp[:, :], wt_hi[:, :], oeh[:, :], start=True, stop=False)
                nc.tensor.matmul(vp[:, :], wt_lo[:, :], oel[:, :], start=False, stop=True)
                V = sb2.tile([128, GB * NCAP], bf16, name=f"V{gi}_{it}", tag="V")
                nc.vector.tensor_copy(out=V[:, :], in_=vp[:, :])
                return V

            # ---------- staged group pipeline, hand-interleaved emission ----------
            st = {gi: {} for gi in range(len(GROUPS))}

            def chainA(gi):
                # iter 1: s = 0.1 * r0_g^T W, squash, V for iter 2
                g0 = GROUPS[gi][0]
                s1p = psS.tile([GB, KND], fp32, name=f"s1p{gi}", tag="sacc")
                nc.tensor.matmul(s1p[:, :], r0[:, g0 : g0 + GB], W_sb[:, :])
                s_g = sb2.tile([GB, KND], fp32, name=f"s1_{gi}", tag="sg")
                nc.vector.tensor_scalar_mul(s_g[:, :], s1p[:, :], 0.1)
                o_g = squash(s_g, 1, gi)
                st[gi]["V"] = make_V(o_g, 2, gi)

            def bp_sfmx(gi, it):
                batches = GROUPS[gi]
                g0 = batches[0]
                V = st[gi]["V"]
                btps = {}
                for b in batches:
                    btp = psB.tile([128, nt * NCAP], fp32, name=f"btp{gi}_{it}_{b}", tag="btp")
                    bl = b - g0
                    for j in range(nt):
                        nc.tensor.matmul(
                            btp[:, NCAP * j : NCAP * (j + 1)],
                            UT_b[b][:, 128 * j : 128 * (j + 1)],
                            V[:, NCAP * bl : NCAP * (bl + 1)],
                        )
                    btps[b] = btp
                ccs = {}
                for b in batches:
                    eb = sb2.tile([128, nt * NCAP], fp32, name=f"eb{it}_{b}", tag="eb")
                    nc.scalar.activation(eb[:, :], btps[b][:, :], ACTF.Exp)
                    ebv = eb[:, :].rearrange("p (j n) -> p j n", j=nt)
                    # Z and cc on GpSimd: it is idle once the load DGE work is
                    # done, while DVE is the post-load bottleneck engine
                    Z = sb2.tile([128, nt], fp32, name=f"Z{it}_{b}", tag="Z")
                    nc.vector.reduce_sum(out=Z[:, :], in_=ebv, axis=AX.X, op=ALU.add)
                    rZ = sb2.tile([128, nt], fp32, name=f"rZ{it}_{b}", tag="rZ")
                    nc.vector.reciprocal(out=rZ[:, :], in_=Z[:, :])
                    cc = sb2.tile([128, nt * NCAP], bf16, name=f"cc{it}_{b}", tag="cc")
                    nc.gpsimd.tensor_tensor(
                        out=cc[:, :].rearrange("p (j n) -> p j n", j=nt),
                        in0=ebv,
                        in1=rZ[:, :].unsqueeze(2).broadcast_to([128, nt, NCAP]),
                        op=ALU.mult,
                    )
                    ccs[b] = cc
                st[gi]["ccs"] = ccs

            def Rpass(gi, it):
                batches = GROUPS[gi]
                g0 = batches[0]
                ccs = st[gi]["ccs"]
                sp = psS.tile([GB, KND], fp32, name=f"sp{gi}_{it}", tag="sacc")
                for b in batches:
                    bl = b - g0
                    cc = ccs[b]
                    Rp = psR.tile([128, NCAP], fp32, name=f"Rp{it}_{b}", tag="Rp")
                    for j in range(nt):
                        nc.tensor.matmul(
                            Rp[:, :],
                            Uv[b][:, j],
                            cc[:, NCAP * j : NCAP * (j + 1)],
                            start=(j == 0),
                            stop=(j == nt - 1),
                        )
                    prod = sb2.tile([128, KND], bf16, name=f"prod{it}_{b}", tag="prod")
                    nc.vector.tensor_tensor(
                        out=prod[:, :].rearrange("p (n d) -> p n d", n=NCAP),
                        in0=Rp[:, :].unsqueeze(2).broadcast_to([128, NCAP, DCAP]),
                        in1=Wv,
                        op=ALU.mult,
                    )
                    nc.tensor.matmul(
                        sp[:, :],
                        esel[:, GB * bl : GB * (bl + 1)],
                        prod[:, :],
                        start=(bl == 0),
                        stop=(bl == GB - 1),
                    )
                s_g = sb2.tile([GB, KND], fp32, name=f"s{gi}_{it}", tag="sg")
                nc.vector.tensor_copy(out=s_g[:, :], in_=sp[:, :])
                st[gi]["s"] = s_g

            def chainB(gi, it):
                o_g = squash(st[gi]["s"], it, gi)
                st[gi]["V"] = make_V(o_g, it + 1, gi)

            def chainC(gi):
                g0 = GROUPS[gi][0]
                o_g = squash(st[gi]["s"], 3, gi)
                nc.sync.dma_start(out=out_h.ap()[g0 : g0 + GB, :], in_=o_g[:, :])

            P, A, CC = phase1, chainA, chainC
            CB = lambda g: chainB(g, 2)
            B2 = lambda g: bp_sfmx(g, 2)
            B3 = lambda g: bp_sfmx(g, 3)
            R2 = lambda g: Rpass(g, 2)
            R3 = lambda g: Rpass(g, 3)
            emit = [
                (P, 0), (P, 1),
                (A, 0),
                (B2, 0), (P, 2), (R2, 0), (P, 3), (CB, 0),
                (B3, 0), (A, 1), (R3, 0), (CC, 0),
                (B2, 1), (P, 4), (R2, 1), (P, 5), (CB, 1),
                (B3, 1), (A, 2), (R3, 1), (CC, 1),
                (B2, 2), (P, 6), (R2, 2), (P, 7), (CB, 2), (A, 3),
                (B3, 2), (B2, 3), (R3, 2), (R2, 3), (CC, 2),
                (CB, 3), (B3, 3), (R3, 3), (CC, 3),
            ]
            for fn, arg in emit:
                fn(arg)

    nc.compile()
    return nc


def make_const_inputs():
    import ml_dtypes

    ident = np.eye(128, dtype=np.float32)
    mask = np.zeros((KND, NCAP), dtype=np.float32)
    for k in range(KND):
        mask[k, k // DCAP] = 1.0
    esel = np.zeros((128, GB * GB), dtype=np.float32)
    for b in range(GB):
        esel[:, b * GB + b] = 1.0
    return {
        "ident": ident,
        "identb": ident.astype(ml_dtypes.bfloat16),
        "m_hi": mask[:128].astype(ml_dtypes.bfloat16),
        "m_lo": mask[128:].astype(ml_dtypes.bfloat16),
        "esel": esel.astype(ml_dtypes.bfloat16),
    }


def make_w_inputs(W):
    import ml_dtypes

    W = np.asarray(W, dtype=np.float32)
    WT = W.T.copy()  # [160, 128]
    return {
        "w": W,
        "wt_hi": WT[:128].astype(ml_dtypes.bfloat16),
        "wt_lo": WT[128:].astype(ml_dtypes.bfloat16),
    }


_CACHE = {}


def kernel(u_vecs, W):
    from concourse import bass_utils

    u_vecs = np.asarray(u_vecs, dtype=np.float32)
    W = np.asarray(W, dtype=np.float32)
    if "nc" not in _CACHE:
        _CACHE["nc"] = build_nc()
    nc = _CACHE["nc"]

    consts = make_const_inputs()
    wis = make_w_inputs(W)
    in_maps = []
    for c in range(NCORES):
        m = {"u": np.ascontiguousarray(u_vecs[c * BC : (c + 1) * BC])}
        m.update(consts)
        m.update(wis)
        in_maps.append(m)

    res = bass_utils.run_bass_kernel_spmd(nc, in_maps, core_ids=list(range(NCORES)))
    outs = [r["out"] for r in res.results]
    return np.concatenate(outs, axis=0).reshape(B, NCAP, DCAP).astype(np.float32)


# revision 29
# speedup vs baseline: 1.1421x; 1.1026x over previous
"""Trainium2 Bass kernel for CapsNet dynamic routing (nn_Capsule_13692355740297).

Math (per batch element):
    u_hat[i, (n,d)] = u[i, :] @ W[:, (n,d)]            # never materialized
    iter1: c uniform 1/10  -> s1 = 0.1 * (sum_i u_i)^T W
    iter k: b[i, n] = v_n . u_i   with v_n = W_n o_n   # contract Din on PE
            c = softmax_n(b)                           # free-dim softmax, [i,n] layout
            R[n, :] = sum_i c[i, n] u_i                # contract i on PE
            s[n, :] = R[n, :] @ W_n                    # small fixup matmuls
            o = squash(s)
Sharding: data-parallel over batch, 8 batch elements per core, no collectives.

v4: batches run in 4 independent groups of 2 whose routing overlaps the HBM
load of later batches.  The Tile scheduler's cost model does not model HBM
contention (it thinks the u loads land in a few us), so the static per-engine
order it produces is essentially program order; stages are therefore emitted
in an order hand-matched to the TRUE load timeline, with next-batch
transposes filling the current group's softmax/squash stalls.  squash's sqrt
runs on the vector engine (rsqrt bit-hack + Newton) so the ACT engine only
ever needs the exp table set (act-table reloads cost ~2.7us each).
"""

import numpy as np

B, I_FULL, DIN = 64, 4096, 128
NCAP, DCAP = 10, 16
KND = NCAP * DCAP  # 160
NCORES = 8
BC = B // NCORES  # 8 batch elements per core
NT_FULL = I_FULL // 128  # 32 i-tiles per batch
EPS = 1e-7
GB = 4  # batches per group
GROUPS = [list(range(g, g + GB)) for g in range(0, BC, GB)]


def build_nc(bc=BC, nt=NT_FULL):
    import concourse.bacc as bacc
    import concourse.mybir as mybir
    from concourse.tile import TileContext

    fp32 = mybir.dt.float32
    bf16 = mybir.dt.bfloat16
    AX = mybir.AxisListType
    ALU = mybir.AluOpType
    ACTF = mybir.ActivationFunctionType

    il = nt * 128  # I per batch

    nc = bacc.Bacc(trn_type="TRN2")
    u_h = nc.dram_tensor("u", [bc, il, DIN], fp32, kind="ExternalInput")
    w_h = nc.dram_tensor("w", [DIN, KND], fp32, kind="ExternalInput")
    ident_h = nc.dram_tensor("ident", [128, 128], fp32, kind="ExternalInput")
    identb_h = nc.dram_tensor("identb", [128, 128], bf16, kind="ExternalInput")
    wt_hi_h = nc.dram_tensor("wt_hi", [128, DIN], bf16, kind="ExternalInput")
    wt_lo_h = nc.dram_tensor("wt_lo", [32, DIN], bf16, kind="ExternalInput")
    m_hi_h = nc.dram_tensor("m_hi", [128, NCAP], bf16, kind="ExternalInput")
    m_lo_h = nc.dram_tensor("m_lo", [32, NCAP], bf16, kind="ExternalInput")
    esel_h = nc.dram_tensor("esel", [128, GB * GB], bf16, kind="ExternalInput")
    out_h = nc.dram_tensor("out", [bc, KND], fp32, kind="ExternalOutput")

    with TileContext(nc) as tc:
        with (
            tc.tile_pool(name="big", bufs=1) as big,
            tc.tile_pool(name="sb2", bufs=3) as sb2,
            tc.tile_pool(name="psT", bufs=2, space="PSUM") as psT,
            tc.tile_pool(name="psB", bufs=3, space="PSUM") as psB,
            tc.tile_pool(name="psR", bufs=2, space="PSUM") as psR,
            tc.tile_pool(name="psS", bufs=1, space="PSUM") as psS,
        ):
            # ---------- persistent SBUF ----------
            U_b = [big.tile([128, il], bf16, name=f"U_sb{b}") for b in range(bc)]
            UT_b = [big.tile([128, il], bf16, name=f"UT_sb{b}") for b in range(bc)]
            W_sb = big.tile([128, KND], fp32, name="W_sb")
            ident = big.tile([128, 128], fp32, name="ident_sb")
            identb = big.tile([128, 128], bf16, name="identb_sb")
            wt_hi = big.tile([128, DIN], bf16, name="wt_hi_sb")
            wt_lo = big.tile([32, DIN], bf16, name="wt_lo_sb")
            m_hi = big.tile([128, NCAP], bf16, name="m_hi_sb")
            m_lo = big.tile([32, NCAP], bf16, name="m_lo_sb")
            esel = big.tile([128, GB * GB], bf16, name="esel_sb")
            r0 = big.tile([128, bc], fp32, name="r0_sb")

            Uv = [
                U_b[b][:, :].rearrange("p (m d) -> p m d", m=nt, d=128)
                for b in range(bc)
            ]
            Wv = W_sb[:, :].rearrange("p (n d) -> p n d", n=NCAP)

            # ---------- u loads first: 16KB contiguous per partition ----------
            hh = nt // 2
            for b in range(bc):
                uin = u_h.ap()[b].rearrange("(p m) d -> p m d", m=nt)
                nc.gpsimd.dma_start(out=Uv[b][:, :hh], in_=uin[:, :hh])
                nc.gpsimd.dma_start(out=Uv[b][:, hh:], in_=uin[:, hh:])

            # ---------- consts on the HWDGE path (parallel with u DGE) ----------
            nc.sync.dma_start(out=W_sb[:, :], in_=w_h.ap())
            nc.sync.dma_start(out=ident[:, :], in_=ident_h.ap())
            nc.sync.dma_start(out=identb[:, :], in_=identb_h.ap())
            nc.sync.dma_start(out=wt_hi[:, :], in_=wt_hi_h.ap())
            nc.sync.dma_start(out=wt_lo[:, :], in_=wt_lo_h.ap())
            nc.sync.dma_start(out=m_hi[:, :], in_=m_hi_h.ap())
            nc.sync.dma_start(out=m_lo[:, :], in_=m_lo_h.ap())
            nc.sync.dma_start(out=esel[:, :], in_=esel_h.ap())

            # ---------- per-batch: UT via PE matmul-transposes, r0 via copy-accumulators ----------
            # regular matmul (identity moving) instead of transpose-mode:
            # the stationary U_j load then qualifies for Fast Weight Load
            # (128 bf16 cols), about 2x cheaper on the PE weight path
            TG = 4
            ng = nt // TG

            def phase1(b):
                racc = sb2.tile([128, ng], fp32, name=f"racc{b}", tag="racc")
                for g in range(ng):
                    tp = psT.tile([128, TG * 128], fp32, name="tp", tag="tp")
                    for jj in range(TG):
                        nc.tensor.matmul(
                            tp[:, 128 * jj : 128 * (jj + 1)],
                            Uv[b][:, TG * g + jj],
                            identb[:, :],
                        )
                    dst = UT_b[b][:, TG * 128 * g : TG * 128 * (g + 1)]
                    if g % 2 == 0:
                        nc.scalar.activation(
                            dst, tp[:, :], ACTF.Copy, accum_out=racc[:, g : g + 1]
                        )
                    else:
                        nc.vector.tensor_scalar(
                            out=dst, in0=tp[:, :],
                            scalar1=0.0, scalar2=0.0,
                            op0=ALU.add, op1=ALU.add,
                            accum_out=racc[:, g : g + 1],
                        )
                nc.vector.reduce_sum(
                    out=r0[:, b : b + 1], in_=racc[:, :], axis=AX.X, op=ALU.add
                )

            def squash(s_g, it, gi):
                """s_g: [GB, KND] sbuf tile -> o_g [GB, KND].

                sqrt via rsqrt bit-hack + one Newton step on DVE; square on
                ACT (Square is in the exp table set, so no table reload).
                """
                i32 = mybir.dt.int32
                sq = sb2.tile([GB, KND], fp32, name=f"sq{gi}_{it}", tag="sq")
                q = sb2.tile([GB, NCAP], fp32, name=f"q{gi}_{it}", tag="q")
                h = sb2.tile([GB, NCAP], fp32, name=f"h{gi}_{it}", tag="h")
                y0 = sb2.tile([GB, NCAP], fp32, name=f"y0{gi}_{it}", tag="y0")
                y1 = sb2.tile([GB, NCAP], fp32, name=f"y1{gi}_{it}", tag="y1")
                a = sb2.tile([GB, NCAP], fp32, name=f"a{gi}_{it}", tag="aa")
                c = sb2.tile([GB, NCAP], fp32, name=f"c{gi}_{it}", tag="cc2")
                rt = sb2.tile([GB, NCAP], fp32, name=f"rt{gi}_{it}", tag="rt")
                den = sb2.tile([GB, NCAP], fp32, name=f"den{gi}_{it}", tag="den")
                rden = sb2.tile([GB, NCAP], fp32, name=f"rden{gi}_{it}", tag="rden")
                coef = sb2.tile([GB, NCAP], fp32, name=f"coef{gi}_{it}", tag="coef")
                o_g = sb2.tile([GB, KND], fp32, name=f"o{gi}_{it}", tag="og")
                nc.scalar.square(out=sq[:, :], in_=s_g[:, :])
                nc.vector.reduce_sum(
                    out=q[:, :],
                    in_=sq[:, :].rearrange("b (n d) -> b n d", n=NCAP),
                    axis=AX.X, op=ALU.add,
                )
                nc.vector.tensor_scalar(
                    out=h[:, :].bitcast(i32), in0=q[:, :].bitcast(i32),
                    scalar1=1, scalar2=None, op0=ALU.arith_shift_right,
                )
                nc.vector.tensor_scalar(
                    out=y0[:, :].bitcast(i32), in0=h[:, :].bitcast(i32),
                    scalar1=-1, scalar2=0x5F3759DF, op0=ALU.mult, op1=ALU.add,
                )
                nc.vector.tensor_tensor(
                    out=a[:, :], in0=y0[:, :], in1=y0[:, :], op=ALU.mult
                )
                nc.vector.tensor_tensor(
                    out=a[:, :], in0=a[:, :], in1=q[:, :], op=ALU.mult
                )
                nc.vector.tensor_scalar(
                    out=c[:, :], in0=a[:, :],
                    scalar1=-0.5, scalar2=1.5, op0=ALU.mult, op1=ALU.add,
                )
                nc.vector.tensor_tensor(
                    out=y1[:, :], in0=y0[:, :], in1=c[:, :], op=ALU.mult
                )
                nc.vector.tensor_tensor(
                    out=rt[:, :], in0=q[:, :], in1=y1[:, :], op=ALU.mult
                )
                nc.vector.tensor_scalar_add(den[:, :], q[:, :], 1.0)
                nc.vector.reciprocal(out=rden[:, :], in_=den[:, :])
                nc.vector.tensor_tensor(
                    out=coef[:, :], in0=rt[:, :], in1=rden[:, :], op=ALU.mult
                )
                nc.vector.tensor_tensor(
                    out=o_g[:, :].rearrange("b (n d) -> b n d", n=NCAP),
                    in0=s_g[:, :].rearrange("b (n d) -> b n d", n=NCAP),
                    in1=coef[:, :].unsqueeze(2).broadcast_to([GB, NCAP, DCAP]),
                    op=ALU.mult,
                )
                return o_g

            def make_V(o_g, it, gi):
                oth_p = psT.tile([128, GB], fp32, name=f"oth{gi}_{it}", tag="tp")
                otl_p = psT.tile([32, GB], fp32, name=f"otl{gi}_{it}", tag="tp")
                nc.tensor.transpose(oth_p[:, :], o_g[:, 0:128], ident[:GB, :GB])
                nc.tensor.transpose(otl_p[:, :], o_g[:, 128:KND], ident[:GB, :GB])
                # oeh/oel read the transposed o straight from PSUM (DVE can),
                # skipping two ACT copies on the critical chain
                oeh = sb2.tile([128, GB * NCAP], bf16, name=f"oeh{gi}_{it}", tag="oeh")
                oel = sb2.tile([32, GB * NCAP], bf16, name=f"oel{gi}_{it}", tag="oel")
                nc.vector.tensor_tensor(
                    out=oeh[:, :].rearrange("p (b n) -> p b n", b=GB),
                    in0=oth_p[:, :].unsqueeze(2).broadcast_to([128, GB, NCAP]),
                    in1=m_hi[:, :].unsqueeze(1).broadcast_to([128, GB, NCAP]),
                    op=ALU.mult,
                )
                nc.vector.tensor_tensor(
                    out=oel[:, :].rearrange("p (b n) -> p b n", b=GB),
                    in0=otl_p[:, :].unsqueeze(2).broadcast_to([32, GB, NCAP]),
                    in1=m_lo[:, :].unsqueeze(1).broadcast_to([32, GB, NCAP]),
                    op=ALU.mult,
                )
                vp = psT.tile([128, GB * NCAP], fp32, name=f"vp{gi}_{it}", tag="tp")
                nc.tensor.matmul(vp[:, :], wt_hi[:, :], oeh[:, :], start=True, stop=False)
                nc.tensor.matmul(vp[:, :], wt_lo[:, :], oel[:, :], start=False, stop=True)
                V = sb2.tile([128, GB * NCAP], bf16, name=f"V{gi}_{it}", tag="V")
                nc.vector.tensor_copy(out=V[:, :], in_=vp[:, :])
                return V

            # ---------- staged group pipeline, hand-interleaved emission ----------
            st = {gi: {} for gi in range(len(GROUPS))}

            def chainA(gi):
                # iter 1: s = 0.1 * r0_g^T W, squash, V for iter 2
                g0 = GROUPS[gi][0]
                s1p = psS.tile([GB, KND], fp32, name=f"s1p{gi}", tag="sacc")
                nc.tensor.matmul(s1p[:, :], r0[:, g0 : g0 + GB], W_sb[:, :])
                s_g = sb2.tile([GB, KND], fp32, name=f"s1_{gi}", tag="sg")
                nc.vector.tensor_scalar_mul(s_g[:, :], s1p[:, :], 0.1)
                o_g = squash(s_g, 1, gi)
                st[gi]["V"] = make_V(o_g, 2, gi)

            def bp_sfmx(gi, it):
                batches = GROUPS[gi]
                g0 = batches[0]
                V = st[gi]["V"]
                btps = {}
                for b in batches:
                    btp = psB.tile([128, nt * NCAP], fp32, name=f"btp{gi}_{it}_{b}", tag="btp")
                    bl = b - g0
                    for j in range(nt):
                        nc.tensor.matmul(
                            btp[:, NCAP * j : NCAP * (j + 1)],
                            UT_b[b][:, 128 * j : 128 * (j + 1)],
                            V[:, NCAP * bl : NCAP * (bl + 1)],
                        )
                    btps[b] = btp
                ccs = {}
                for b in batches:
                    eb = sb2.tile([128, nt * NCAP], fp32, name=f"eb{it}_{b}", tag="eb")
                    nc.scalar.activation(eb[:, :], btps[b][:, :], ACTF.Exp)
                    ebv = eb[:, :].rearrange("p (j n) -> p j n", j=nt)
                    # Z and cc on GpSimd: it is idle once the load DGE work is
                    # done, while DVE is the post-load bottleneck engine
                    Z = sb2.tile([128, nt], fp32, name=f"Z{it}_{b}", tag="Z")
                    nc.vector.reduce_sum(out=Z[:, :], in_=ebv, axis=AX.X, op=ALU.add)
                    rZ = sb2.tile([128, nt], fp32, name=f"rZ{it}_{b}", tag="rZ")
                    nc.vector.reciprocal(out=rZ[:, :], in_=Z[:, :])
                    cc = sb2.tile([128, nt * NCAP], bf16, name=f"cc{it}_{b}", tag="cc")
                    nc.gpsimd.tensor_tensor(
                        out=cc[:, :].rearrange("p (j n) -> p j n", j=nt),
                        in0=ebv,
                        in1=rZ[:, :].unsqueeze(2).broadcast_to([128, nt, NCAP]),
                        op=ALU.mult,
                    )
                    ccs[b] = cc
                st[gi]["ccs"] = ccs

            def Rpass(gi, it):
                batches = GROUPS[gi]
                g0 = batches[0]
                ccs = st[gi]["ccs"]
                sp = psS.tile([GB, KND], fp32, name=f"sp{gi}_{it}", tag="sacc")
                for b in batches:
                    bl = b - g0
                    cc = ccs[b]
                    Rp = psR.tile([128, NCAP], fp32, name=f"Rp{it}_{b}", tag="Rp")
                    for j in range(nt):
                        nc.tensor.matmul(
                            Rp[:, :],
                            Uv[b][:, j],
                            cc[:, NCAP * j : NCAP * (j + 1)],
                            start=(j == 0),
                            stop=(j == nt - 1),
                        )
                    prod = sb2.tile([128, KND], bf16, name=f"prod{it}_{b}", tag="prod")
                    nc.vector.tensor_tensor(
                        out=prod[:, :].rearrange("p (n d) -> p n d", n=NCAP),
                        in0=Rp[:, :].unsqueeze(2).broadcast_to([128, NCAP, DCAP]),
                        in1=Wv,
                        op=ALU.mult,
                    )
                    nc.tensor.matmul(
                        sp[:, :],
                        esel[:, GB * bl : GB * (bl + 1)],
                        prod[:, :],
                        start=(bl == 0),
                        stop=(bl == GB - 1),
                    )
                s_g = sb2.tile([GB, KND], fp32, name=f"s{gi}_{it}", tag="sg")
                nc.vector.tensor_copy(out=s_g[:, :], in_=sp[:, :])
                st[gi]["s"] = s_g

            def chainB(gi, it):
                o_g = squash(st[gi]["s"], it, gi)
                st[gi]["V"] = make_V(o_g, it + 1, gi)

            def chainC(gi):
                g0 = GROUPS[gi][0]
                o_g = squash(st[gi]["s"], 3, gi)
                nc.sync.dma_start(out=out_h.ap()[g0 : g0 + GB, :], in_=o_g[:, :])

            P, A, CC = phase1, chainA, chainC
            CB = lambda g: chainB(g, 2)
            B2 = lambda g: bp_sfmx(g, 2)
            B3 = lambda g: bp_sfmx(g, 3)
            R2 = lambda g: Rpass(g, 2)
            R3 = lambda g: Rpass(g, 3)
            emit = [
                (P, 0), (P, 1), (P, 2), (P, 3),
                (A, 0),
                (B2, 0), (P, 4), (R2, 0), (P, 5), (CB, 0),
                (B3, 0), (P, 6), (R3, 0), (P, 7), (CC, 0),
                (A, 1),
                (B2, 1), (R2, 1), (CB, 1),
                (B3, 1), (R3, 1), (CC, 1),
            ]
            for fn, arg in emit:
                fn(arg)

    nc.compile()
    return nc


def make_const_inputs():
    import ml_dtypes

    ident = np.eye(128, dtype=np.float32)
    mask = np.zeros((KND, NCAP), dtype=np.float32)
    for k in range(KND):
        mask[k, k // DCAP] = 1.0
    esel = np.zeros((128, GB * GB), dtype=np.float32)
    for b in range(GB):
        esel[:, b * GB + b] = 1.0
    return {
        "ident": ident,
        "identb": ident.astype(ml_dtypes.bfloat16),
        "m_hi": mask[:128].astype(ml_dtypes.bfloat16),
        "m_lo": mask[128:].astype(ml_dtypes.bfloat16),
        "esel": esel.astype(ml_dtypes.bfloat16),
    }


def make_w_inputs(W):
    import ml_dtypes

    W = np.asarray(W, dtype=np.float32)
    WT = W.T.copy()  # [160, 128]
    return {
        "w": W,
        "wt_hi": WT[:128].astype(ml_dtypes.bfloat16),
        "wt_lo": WT[128:].astype(ml_dtypes.bfloat16),
    }


_CACHE = {}


def kernel(u_vecs, W):
    from concourse import bass_utils

    u_vecs = np.asarray(u_vecs, dtype=np.float32)
    W = np.asarray(W, dtype=np.float32)
    if "nc" not in _CACHE:
        _CACHE["nc"] = build_nc()
    nc = _CACHE["nc"]

    consts = make_const_inputs()
    wis = make_w_inputs(W)
    in_maps = []
    for c in range(NCORES):
        m = {"u": np.ascontiguousarray(u_vecs[c * BC : (c + 1) * BC])}
        m.update(consts)
        m.update(wis)
        in_maps.append(m)

    res = bass_utils.run_bass_kernel_spmd(nc, in_maps, core_ids=list(range(NCORES)))
    outs = [r["out"] for r in res.results]
    return np.concatenate(outs, axis=0).reshape(B, NCAP, DCAP).astype(np.float32)
